# revision 13
# baseline (speedup 1.0000x reference)
"""Causal multi-head self-attention (B=4, S=2048, D=1024, 16 heads) on 8 TRN2 cores.

Sharding: core c -> batch b = c//2, head-half hh = c%2 (8 of 16 heads, 512 of
1024 projection dims).  All activations/weights bf16 (host-cast); measured
engine rates drive the design:
  - q/k projections in scores-transposed layout (dims on partitions) with
    weight-stationary matmul ordering (one [128,2048] 4-bank psum per output
    block, bias+cast drained by DVE tensor_scalar_add at 1024 width),
  - v projection natural layout (x sub-blocks stationary, wv moving), bias via
    a K=1 ones matmul, softmax denominators via an appended ones-column in v,
  - attention in scores-transposed layout: head pairs share one kT/qT tile at
    partition offsets 0/64 (concurrent PE row groups); exp on ScalarE as two
    contiguous 512-wide single-bank PSUM reads (fast path, 213ns/unit);
    diagonal masks multiplied in bf16 on DVE; normalization by a K=1
    outer-product broadcast of 1/sums, all-bf16 multiply,
  - output projection TRANSPOSED (d_model on partitions): wo blocks stationary,
    ctx chunks moving, bias bo/2 via K=1 ones matmul; bf16 partials to DRAM,
  - per-512-token-chunk pairwise ReduceScatter (bf16) splits d_model rows:
    even core ends with dm 0:512, odd with dm 512:1024, for all its tokens.
Host assembles/transposes/casts to f32.
"""

import numpy as np

B = 4
S = 2048
DM = 1024
HD = 64
NH = 8            # heads per core
OD = NH * HD      # 512: per-core projection width
NCORES = 8
QCN = S // 512    # 4 query chunks of 512
DBLK = DM // 128  # 8 contraction blocks
OBLK = OD // 128  # 4 output-dim blocks
SBLK = S // 128   # 16 seq strips

_prog = None


def _build_program(repeat=1):
    from contextlib import ExitStack
    from concourse import bacc, mybir
    import concourse.tile as tile

    f32 = mybir.dt.float32
    f32r = mybir.dt.float32r
    bf16 = mybir.dt.bfloat16
    EXP = mybir.ActivationFunctionType.Exp

    nc = bacc.Bacc(None, num_devices=NCORES)

    # --- external I/O (per-core shards) ---
    xq_ext = nc.declare_dram_parameter("xq", [DM, S], bf16, isOutput=False)
    xk_ext = nc.declare_dram_parameter("xk", [DM, S], bf16, isOutput=False)
    xv_ext = nc.declare_dram_parameter("xv", [DM, S], bf16, isOutput=False)
    wq_ext = nc.declare_dram_parameter("wq", [DM, OD], bf16, isOutput=False)
    wk_ext = nc.declare_dram_parameter("wk", [DM, OD], bf16, isOutput=False)
    wv_ext = nc.declare_dram_parameter("wv", [DM, OD], bf16, isOutput=False)
    wo_ext = nc.declare_dram_parameter("wo", [OD, DM], bf16, isOutput=False)
    bq_ext = nc.declare_dram_parameter("bq2", [128, OBLK], f32, isOutput=False)
    bk_ext = nc.declare_dram_parameter("bk2", [128, OBLK], f32, isOutput=False)
    bv_ext = nc.declare_dram_parameter("bvr", [1, OD], bf16, isOutput=False)
    bo_ext = nc.declare_dram_parameter("bo2", [1, DM], bf16, isOutput=False)
    mask_ext = nc.declare_dram_parameter("mask", [128, 128], bf16, isOutput=False)
    out_ext = nc.declare_dram_parameter("out", [OD, S], bf16, isOutput=True)

    with tile.TileContext(nc) as tc, ExitStack() as ctx:
        consts = ctx.enter_context(tc.tile_pool(name="consts", bufs=1))
        persist = ctx.enter_context(tc.tile_pool(name="persist", bufs=1))
        xpool = ctx.enter_context(tc.tile_pool(name="xpool", bufs=1))
        wpool = ctx.enter_context(tc.tile_pool(name="wpool", bufs=1))
        ppool = ctx.enter_context(tc.tile_pool(name="ppool", bufs=4))
        stg = ctx.enter_context(tc.tile_pool(name="stg", bufs=2))
        ps_mm = ctx.enter_context(tc.tile_pool(name="ps_mm", bufs=2, space="PSUM"))
        ps_acc = ctx.enter_context(tc.tile_pool(name="ps_acc", bufs=1, space="PSUM"))
        ps_tr = ctx.enter_context(tc.tile_pool(name="ps_tr", bufs=1, space="PSUM"))
        dram = ctx.enter_context(tc.tile_pool(name="dram", bufs=1, space="DRAM"))

        # --- constants ---
        mask_sb = consts.tile([128, 128], bf16, name="mask_sb")
        bq_sb = consts.tile([128, OBLK], f32, name="bq_sb")
        bk_sb = consts.tile([128, OBLK], f32, name="bk_sb")
        bv_sb = consts.tile([1, OD], bf16, name="bv_sb")
        bo_sb = consts.tile([1, DM], bf16, name="bo_sb")
        ones1r = consts.tile([1, 128], f32r, name="ones1r")
        ones1b = consts.tile([1, 128], bf16, name="ones1b")
        ones512 = consts.tile([1, 512], bf16, name="ones512")
        ones_col = consts.tile([128, NH, 1], bf16, name="ones_col")
        nc.sync.dma_start(out=mask_sb, in_=mask_ext[:, :])
        nc.sync.dma_start(out=bq_sb, in_=bq_ext[:, :])
        nc.sync.dma_start(out=bk_sb, in_=bk_ext[:, :])
        nc.sync.dma_start(out=bv_sb, in_=bv_ext[:, :])
        nc.sync.dma_start(out=bo_sb, in_=bo_ext[:, :])
        ones512f = consts.tile([1, 512], f32, name="ones512f")
        ones_colf = consts.tile([128, NH, 1], f32, name="ones_colf")
        nc.vector.memset(ones512f, 1.0)
        nc.vector.memset(ones_colf, 1.0)
        nc.vector.tensor_copy(out=ones1r, in_=ones512f[:, 0:128])
        nc.vector.tensor_copy(out=ones1b, in_=ones512f[:, 0:128])
        nc.vector.tensor_copy(out=ones512, in_=ones512f)
        nc.vector.tensor_copy(out=ones_col, in_=ones_colf)

        # --- persistent activations (bf16) ---
        qT = [persist.tile([128, S], bf16, name=f"qT{i}") for i in range(OBLK)]
        kT = [persist.tile([128, S], bf16, name=f"kT{i}") for i in range(OBLK)]
        # v_sb[s]: [128, 8 heads * 65]; col 65h+64 is the ones column
        v_sb = [persist.tile([128, NH * (HD + 1)], bf16, name=f"v{s}") for s in range(SBLK)]
        ctxT = [persist.tile([128, S], bf16, name=f"ctxT{i}") for i in range(OBLK)]

        # --- weights (bf16): wq/wk/wv as 8 d-tiles, wo as 4 head-pair tiles ---
        wq_sb = [wpool.tile([128, OD], bf16, name=f"wq{d}") for d in range(DBLK)]
        wk_sb = [wpool.tile([128, OD], bf16, name=f"wk{d}") for d in range(DBLK)]
        wv_sb = [wpool.tile([128, OD], bf16, name=f"wv{d}") for d in range(DBLK)]
        wo_sb = [wpool.tile([128, DM], bf16, name=f"wo{i}") for i in range(OBLK)]
        for d in range(DBLK):
            nc.sync.dma_start(out=wq_sb[d], in_=wq_ext[d * 128:(d + 1) * 128, :])
        for d in range(DBLK):
            nc.sync.dma_start(out=wk_sb[d], in_=wk_ext[d * 128:(d + 1) * 128, :])
        for d in range(DBLK):
            nc.sync.dma_start(out=wv_sb[d], in_=wv_ext[d * 128:(d + 1) * 128, :])
        for i in range(OBLK):
            nc.sync.dma_start(out=wo_sb[i], in_=wo_ext[i * 128:(i + 1) * 128, :])

        # ones columns of v
        for s in range(SBLK):
            v3 = v_sb[s].rearrange("p (h e) -> p h e", e=HD + 1)
            nc.vector.tensor_copy(out=v3[:, :, HD:HD + 1], in_=ones_col)

        # x tiles: one tensor at a time through xpool (8 d-tiles of [128, S])
        def load_x(x_ext, nm):
            ts = []
            for d in range(DBLK):
                t = xpool.tile([128, S], bf16, name=f"{nm}{d}", tag=f"x{d}")
                nc.sync.dma_start(out=t, in_=x_ext[d * 128:(d + 1) * 128, :])
                ts.append(t)
            return ts

        cc_in = [dram.tile([DM, 512], bf16, name=f"cc_in{qc}") for qc in range(QCN)]
        cc_out = [dram.tile([OD, 512], bf16, name=f"cc_out{qc}") for qc in range(QCN)]

        def _body(rep):
            # --- q/k projections: dst[ob][:, s] = (W.T @ x.T + b), bf16 out ---
            # weight-stationary: per ob, two [128, 1024] psums (4 seq chunks)
            # are live so the w tile (d, ob) stays stationary for 4 matmuls.
            def proj_qk(xt, w_tiles, bias_sb, dst, nm):
                for ob in range(OBLK):
                    pA = ps_mm.tile([128, 1024], f32, name=f"{nm}psA{rep}_{ob}", tag="mm")
                    pB = ps_mm.tile([128, 1024], f32, name=f"{nm}psB{rep}_{ob}", tag="mm")
                    for d in range(DBLK):
                        for sc in range(QCN):
                            ps = pA if sc < 2 else pB
                            nc.tensor.matmul(
                                ps[:, (sc % 2) * 512:(sc % 2 + 1) * 512],
                                w_tiles[d][:, ob * 128:(ob + 1) * 128],
                                xt[d][:, sc * 512:(sc + 1) * 512],
                                start=(d == 0), stop=(d == DBLK - 1))
                    for half, ps in ((0, pA), (1, pB)):
                        nc.vector.tensor_scalar_add(
                            out=dst[ob][:, half * 1024:(half + 1) * 1024],
                            in0=ps,
                            scalar1=bias_sb[:, ob:ob + 1])

            xq_t = load_x(xq_ext, f"xq{rep}_")
            proj_qk(xq_t, wq_sb, bq_sb, qT, "q")
            xk_t = load_x(xk_ext, f"xk{rep}_")
            proj_qk(xk_t, wk_sb, bk_sb, kT, "k")

            # --- v projection: v[s, o] = x @ Wv + bv (natural layout) ---
            xv_t = load_x(xv_ext, f"xv{rep}_")
            for sc in range(QCN):
                for sl in range(4):
                    s = sc * 4 + sl
                    psum = ps_mm.tile([128, 512], f32, name=f"vps{rep}_{s}", tag="mm")
                    for d in range(DBLK):
                        nc.tensor.matmul(
                            psum, xv_t[d][:, s * 128:(s + 1) * 128], wv_sb[d],
                            start=(d == 0), stop=False)
                    nc.tensor.matmul(psum, ones1b, bv_sb, start=False, stop=True)
                    v3 = v_sb[s].rearrange("p (h e) -> p h e", e=HD + 1)
                    ps3 = psum.rearrange("p (h e) -> p h e", e=HD)
                    nc.vector.tensor_copy(out=v3[:, :, 0:HD], in_=ps3)

            # --- attention: chunk by chunk; head pairs in PE row groups ---
            def ctx_mm(acc, h, nkb, qc, pkb, pp):
                m = pkb - 4 * qc
                c0 = 128 * m if m > 0 else 0
                nc.tensor.matmul(
                    acc[:, c0:512], v_sb[pkb][:, 65 * h:65 * h + 65], pp[:, c0:512],
                    start=(pkb == 0), stop=(pkb == nkb - 1))

            def attn_chunk(qc):
                for hp in range(NH // 2):
                    t = hp
                    heads = (2 * hp, 2 * hp + 1)
                    nkb = 4 * qc + 4
                    lq = [qT[t][64 * j:64 * j + 64, qc * 512:(qc + 1) * 512]
                          for j in range(2)]
                    acc2 = ps_acc.tile([HD + 1, 1024], f32,
                                       name=f"acc{rep}_{qc}_{hp}", tag="acc")
                    accs = [acc2[:, 512 * j:512 * (j + 1)] for j in range(2)]
                    pending = []
                    for kb in range(nkb):
                        m = kb - 4 * qc
                        c0 = 128 * m if m > 0 else 0
                        sps = ps_mm.tile([128, 1024], f32,
                                          name=f"s{rep}_{qc}_{hp}_{kb}", tag="mm")
                        for j in range(2):
                            nc.tensor.matmul(
                                sps[:, 512 * j + c0:512 * (j + 1)],
                                kT[t][64 * j:64 * j + 64, kb * 128:(kb + 1) * 128],
                                lq[j][:, c0:512], start=True, stop=True)
                        p = ppool.tile([128, 1024], bf16,
                                       name=f"p{rep}_{qc}_{hp}_{kb}", tag="p")
                        if c0 == 0:
                            nc.scalar.activation(out=p, in_=sps, func=EXP)
                        else:
                            p3 = p.rearrange("k (g q) -> k g q", q=512)
                            s3 = sps.rearrange("k (g q) -> k g q", q=512)
                            nc.scalar.activation(
                                out=p3[:, :, c0:512], in_=s3[:, :, c0:512], func=EXP)
                        if m >= 0:
                            p3m = p.rearrange("k (g q) -> k g q", q=512)
                            msk3 = mask_sb.rearrange("k (g q) -> k g q", g=1)
                            nc.vector.tensor_mul(
                                out=p3m[:, :, 128 * m:128 * (m + 1)],
                                in0=p3m[:, :, 128 * m:128 * (m + 1)],
                                in1=msk3.broadcast_to([128, 2, 128]))
                        pending.append((kb, p))
                        while len(pending) > 2:
                            pkb, pp = pending.pop(0)
                            for j in range(2):
                                ctx_mm(accs[j], heads[j], nkb, qc, pkb,
                                       pp[:, 512 * j:512 * (j + 1)])
                    while pending:
                        pkb, pp = pending.pop(0)
                        for j in range(2):
                            ctx_mm(accs[j], heads[j], nkb, qc, pkb,
                                   pp[:, 512 * j:512 * (j + 1)])

                    # normalize: ctxT[h] = acc[0:64] * broadcast(1/acc[64])
                    # single wide [65,1024] pass covers both heads
                    cstg = stg.tile([HD + 1, 1024], bf16,
                                    name=f"cstg{rep}_{qc}_{hp}", tag="cstg")
                    nc.vector.tensor_copy(out=cstg, in_=acc2)
                    rrow = stg.tile([1, 1024], f32r, name=f"rr{rep}_{qc}_{hp}",
                                    tag="rrow", bufs=4)
                    with nc.allow_low_precision(reason="f32r is fp32-width"):
                        nc.vector.reciprocal(out=rrow, in_=acc2[HD:HD + 1, :])
                    bc = ps_tr.tile([HD, 1024], f32,
                                    name=f"bc{rep}_{qc}_{hp}", tag="tr")
                    for j in range(2):
                        nc.tensor.matmul(bc[:, 512 * j:512 * (j + 1)],
                                         ones1r[:, 0:HD],
                                         rrow[:, 512 * j:512 * (j + 1)],
                                         start=True, stop=True)
                    bcs = stg.tile([HD, 1024], bf16,
                                   name=f"bcs{rep}_{qc}_{hp}", tag="bcs")
                    nc.vector.tensor_copy(out=bcs, in_=bc)
                    for j in range(2):
                        nc.vector.tensor_mul(
                            out=ctxT[t][64 * j:64 * j + 64, qc * 512:(qc + 1) * 512],
                            in0=cstg[0:HD, 512 * j:512 * (j + 1)],
                            in1=bcs[:, 512 * j:512 * (j + 1)])

            def outproj(qc):
                # --- transposed output projection for this 512-token chunk:
                # out[dm, tok] = sum_hb wo[hb].T @ ctx[hb] + bo/2 ---
                for dbp in range(DBLK // 2):
                    psum = ps_mm.tile([128, 1024], f32,
                                       name=f"ops{rep}_{qc}_{dbp}", tag="mm")
                    for half in range(2):
                        dmb = 2 * dbp + half
                        sl = slice(512 * half, 512 * (half + 1))
                        for hb in range(OBLK):
                            nc.tensor.matmul(
                                psum[:, sl],
                                wo_sb[hb][:, dmb * 128:(dmb + 1) * 128],
                                ctxT[hb][:, qc * 512:(qc + 1) * 512],
                                start=(hb == 0), stop=False)
                        nc.tensor.matmul(
                            psum[:, sl], bo_sb[:, dmb * 128:(dmb + 1) * 128],
                            ones512, start=False, stop=True)
                    osb = stg.tile([128, 1024], bf16, name=f"ob{rep}_{qc}_{dbp}",
                                   tag="osb", bufs=3)
                    nc.vector.tensor_copy(out=osb, in_=psum)
                    for half in range(2):
                        dmb = 2 * dbp + half
                        nc.sync.dma_start(
                            out=cc_in[qc][dmb * 128:(dmb + 1) * 128, :],
                            in_=osb[:, 512 * half:512 * (half + 1)])
                nc.gpsimd.collective_compute(
                    "ReduceScatter", mybir.AluOpType.add,
                    replica_groups=[[0, 1], [2, 3], [4, 5], [6, 7]],
                    ins=[cc_in[qc].opt()],
                    outs=[cc_out[qc].opt()])
                nc.sync.dma_start(
                    out=out_ext[:, qc * 512:(qc + 1) * 512],
                    in_=cc_out[qc])
            for qc in range(QCN):
                attn_chunk(qc)
                outproj(qc)

        # repeat>1 is a timing aid: one dispatch runs the body N times
        for _rep in range(repeat):
            _body(_rep)

    nc.finalize()
    return nc


def _get_program(repeat=1):
    global _prog
    if repeat != 1:
        return _build_program(repeat)
    if _prog is None:
        _prog = _build_program()
    return _prog


def make_in_maps(query, key, value, Wq, bq, Wk, bk, Wv, bv, Wo, bo):
    from concourse import mybir
    bfdt = mybir.dt.np(mybir.dt.bfloat16)

    query = np.asarray(query, dtype=np.float32)
    key = np.asarray(key, dtype=np.float32)
    value = np.asarray(value, dtype=np.float32)
    Wq = np.asarray(Wq, dtype=np.float32)
    bq = np.asarray(bq, dtype=np.float32)
    Wk = np.asarray(Wk, dtype=np.float32)
    bk = np.asarray(bk, dtype=np.float32)
    Wv = np.asarray(Wv, dtype=np.float32)
    bv = np.asarray(bv, dtype=np.float32)
    Wo = np.asarray(Wo, dtype=np.float32)
    bo = np.asarray(bo, dtype=np.float32)

    scale = 1.0 / np.sqrt(np.float32(DM))
    mask = np.triu(np.ones((128, 128), dtype=np.float32)).astype(bfdt)

    xq_t = [np.ascontiguousarray(query[b].T).astype(bfdt) for b in range(B)]
    xk_t = [np.ascontiguousarray(key[b].T).astype(bfdt) for b in range(B)]
    xv_t = [np.ascontiguousarray(value[b].T).astype(bfdt) for b in range(B)]

    in_maps = []
    for c in range(NCORES):
        b, hh = c // 2, c % 2
        cols = slice(hh * OD, (hh + 1) * OD)
        in_maps.append({
            "xq": xq_t[b],
            "xk": xk_t[b],
            "xv": xv_t[b],
            "wq": np.ascontiguousarray(Wq[:, cols] * scale).astype(bfdt),
            "wk": np.ascontiguousarray(Wk[:, cols]).astype(bfdt),
            "wv": np.ascontiguousarray(Wv[:, cols]).astype(bfdt),
            "wo": np.ascontiguousarray(Wo[cols, :]).astype(bfdt),
            "bq2": np.ascontiguousarray((bq[cols] * scale).reshape(OBLK, 128).T),
            "bk2": np.ascontiguousarray(bk[cols].reshape(OBLK, 128).T),
            "bvr": np.ascontiguousarray(bv[cols].reshape(1, OD)).astype(bfdt),
            "bo2": np.ascontiguousarray((bo / 2.0).reshape(1, DM)).astype(bfdt),
            "mask": mask,
        })
    return in_maps


def assemble(core_outs):
    """core_outs[c]: [OD, S] bf16 = d_model rows (even core: dm 0:512, odd:
    dm 512:1024) for all its batch's tokens."""
    out = np.empty((B, S, DM), dtype=np.float32)
    for b in range(B):
        ev = np.asarray(core_outs[2 * b], dtype=np.float32)
        od = np.asarray(core_outs[2 * b + 1], dtype=np.float32)
        out[b, :, 0:OD] = ev.T
        out[b, :, OD:DM] = od.T
    return out


def kernel(query, key, value, Wq, bq, Wk, bk, Wv, bv, Wo, bo):
    import time
    from concourse.bass_utils import run_bass_kernel_spmd

    in_maps = make_in_maps(query, key, value, Wq, bq, Wk, bk, Wv, bv, Wo, bo)
    nc = _get_program()
    try:
        res = run_bass_kernel_spmd(nc, in_maps, list(range(NCORES)))
    except Exception:
        time.sleep(10)  # transient device errors recover on retry
        res = run_bass_kernel_spmd(nc, in_maps, list(range(NCORES)))
    return assemble([res.results[c]["out"] for c in range(NCORES)])

OUT_SHAPE = (OD, S)
